# revision 8
# baseline (speedup 1.0000x reference)
"""BTSP memory-bank retrieval kernel for 8 Trainium2 NeuronCores.

Problem: query (4096,1024) f32, memory (65536,1024) f32 unit-norm rows,
W_key (1024,1024) f32, top_k=32.
  query_proj = query @ W_key.T ; qn = l2norm(query_proj)
  sim = qn @ memory.T ; top_sim, top_idx = top_k(sim, 32)
  weights = softmax(top_sim) ; retrieved = weights . memory[top_idx]
Returns (retrieved (4096,1024) f32, top_sim (4096,32) f32).

Sharding: pure data-parallel over queries (B) — each of the 8 cores handles
512 queries against the full memory bank; no collectives. Inside one core:
  - fp32 matmul (PE) computes sim in 512-column tiles, streamed over N
  - per-row chunk maxima (chunks of 128) feed a top-32-chunks tournament:
    the 32nd-largest chunk-max is a valid threshold (32 distinct elements
    are >= it), so the true top-32 elements all live in the 32 selected
    chunks; gather those 4096 candidates and take an exact top-32 with the
    DVE max8/max_index/match_replace instructions
  - softmax over the 32 values, indirect-DMA gather of the 32 memory rows,
    weighted sum -> retrieved.
"""
import numpy as np
import concourse.bass as bass
import concourse.mybir as mybir
from concourse.tile import TileContext
from concourse import bass_utils
from concourse.masks import make_identity

P = 128          # partitions
D = 1024         # feature dim
N = 65536        # memory slots
K = 32           # top-k
BC = 512         # queries per core
NQB = BC // P    # query blocks per core (4)
NT = 512         # sim tile width (N columns per PSUM tile)
NTILES = N // NT # 128
CH = 128         # chunk width for chunk-max tournament
CPT = NT // CH   # chunks per sim tile (4)
NCHUNK = N // CH # chunks per row (512)
NCORES = 8
NEG = -1e30

_cache = {}


def _split_waits(nc, limit=1):
    """Walrus in this container accepts at most `limit` semaphore waits per
    instruction; move excess waits onto preceding same-engine NOPs."""
    n_split = 0
    for f in nc.m.functions:
        for bb in f.blocks:
            new_list = []
            changed = False
            for ins in bb.instructions:
                si = getattr(ins, "sync_info", None)
                waits = list(si.on_wait) if (si is not None and si.on_wait) else []
                if len(waits) > limit:
                    changed = True
                    n_split += 1
                    extra, keep = waits[:-limit], waits[-limit:]
                    for j in range(0, len(extra), limit):
                        nop = mybir.InstNoOp(
                            name=nc.get_next_instruction_name(), ins=[], outs=[]
                        )
                        nop.engine = ins.engine
                        nop.sync_info = mybir.SyncInfo(
                            on_wait=extra[j : j + limit], on_update=[]
                        )
                        new_list.append(nop)
                    si.on_wait = keep
                new_list.append(ins)
            if changed:
                bb.instructions[:] = new_list
    return n_split


def _build():
    f32 = mybir.dt.float32
    nc = bass.Bass(trn_type="TRN2")

    qT = nc.dram_tensor("qT", [D, BC], f32, kind="ExternalInput")        # query shard, transposed
    WT = nc.dram_tensor("WT", [D, D], f32, kind="ExternalInput")         # W_key.T ([din, dout])
    memT = nc.dram_tensor("memT", [D, N], f32, kind="ExternalInput")     # memory transposed
    mem = nc.dram_tensor("mem", [N, D], f32, kind="ExternalInput")       # memory (row gather)

    retr_out = nc.dram_tensor("retrieved", [BC, D], f32, kind="ExternalOutput")
    tsim_out = nc.dram_tensor("top_sim", [BC, K], f32, kind="ExternalOutput")

    with TileContext(nc) as tc:
        with (
            tc.tile_pool(name="persist", bufs=1) as pp,
            tc.tile_pool(name="dram", bufs=1, space="DRAM") as dp,
        ):
            # ---------------- phase 0: query proj + l2norm + transpose -----
            qnT = pp.tile([P, NQB, 8, P], f32)     # lhsT tiles [128d x 128q] per (qb, ktile)
            ident = pp.tile([P, P], f32)
            make_identity(nc, ident[:])
            M = pp.tile([P, NQB, NCHUNK], f32)     # chunk maxima per query block

            with (
                tc.tile_pool(name="proj_sb", bufs=2) as sb0,
                tc.tile_pool(name="proj_ps", bufs=2, space="PSUM") as ps0,
                tc.tile_pool(name="tr_ps", bufs=2, space="PSUM") as ps0t,
            ):
                qt_all = sb0.tile([P, 8, NQB, P], f32, tag="qt")   # [din128, k, qb, q]
                nc.sync.dma_start(
                    qt_all[:], qT[:].rearrange("(k p) (qb q) -> p k qb q", p=P, q=P)
                )
                wt_all = sb0.tile([P, 8, D], f32, tag="wt")        # [din128, k, dout]
                nc.sync.dma_start(wt_all[:], WT[:].rearrange("(k p) d -> p k d", p=P))

                for qb in range(NQB):
                    qp = sb0.tile([P, D], f32, tag="qp")           # [q, dout]
                    for half in range(2):
                        psq = ps0.tile([P, NT], f32, space="PSUM", tag="psq")
                        for k in range(8):
                            nc.tensor.matmul(
                                psq[:],
                                qt_all[:, k, qb, :],
                                wt_all[:, k, half * NT : (half + 1) * NT],
                                start=(k == 0),
                                stop=(k == 7),
                            )
                        nc.vector.tensor_copy(qp[:, half * NT : (half + 1) * NT], psq[:])
                    sq = sb0.tile([P, D], f32, tag="sq")
                    nc.vector.tensor_tensor(out=sq[:], in0=qp[:], in1=qp[:], op=mybir.AluOpType.mult)
                    nrm2 = sb0.tile([P, 1], f32, tag="n2")
                    nc.vector.tensor_reduce(out=nrm2[:], in_=sq[:], axis=mybir.AxisListType.X, op=mybir.AluOpType.add)
                    nrm = sb0.tile([P, 1], f32, tag="nr")
                    nc.scalar.activation(out=nrm[:], in_=nrm2[:], func=mybir.ActivationFunctionType.Sqrt)
                    nc.vector.tensor_scalar_max(nrm[:], nrm[:], 1e-12)
                    rn = sb0.tile([P, 1], f32, tag="rn")
                    nc.vector.reciprocal(rn[:], nrm[:])
                    qn = sb0.tile([P, D], f32, tag="qn")
                    nc.scalar.activation(out=qn[:], in_=qp[:], func=mybir.ActivationFunctionType.Copy, scale=rn[:, 0:1])
                    # transpose 128x128 blocks: qnT[:, qb, k, :] = qn[:, k*128:(k+1)*128].T
                    for k in range(8):
                        pst = ps0t.tile([P, P], f32, space="PSUM", tag="pst")
                        nc.tensor.transpose(pst[:], qn[:, k * P : (k + 1) * P], ident[:])
                        nc.vector.tensor_copy(qnT[:, qb, k, :], pst[:])

            # ---------------- phase 1: sim matmul + chunk maxima -----------
            # one DRAM scratch per query block: indirect-DMA sources need offset 0
            sim_dram = [
                dp.tile([P * NCHUNK, CH], f32, name=f"sim_scratch{i}", tag=f"sim_scratch{i}")
                for i in range(NQB)
            ]
            with (
                tc.tile_pool(name="mem_sb", bufs=3) as mp,
                tc.tile_pool(name="sim_sb", bufs=6) as sp,
                tc.tile_pool(name="sim_ps", bufs=2, space="PSUM") as psp,
            ):
                for nt in range(NTILES):
                    mt = mp.tile([P, 8, NT], f32, tag="mt")
                    nc.sync.dma_start(
                        mt[:], memT[:, nt * NT : (nt + 1) * NT].rearrange("(k p) n -> p k n", p=P)
                    )
                    for qb in range(NQB):
                        ps = psp.tile([P, NT], f32, space="PSUM", tag=f"ps{qb}")
                        for k in range(8):
                            nc.tensor.matmul(
                                ps[:], qnT[:, qb, k, :], mt[:, k, :],
                                start=(k == 0), stop=(k == 7),
                            )
                        sim_sb = sp.tile([P, NT], f32, tag="sim")
                        nc.vector.tensor_copy(sim_sb[:], ps[:])
                        # chunk maxima -> M
                        nc.vector.tensor_reduce(
                            out=M[:, qb, nt * CPT : (nt + 1) * CPT],
                            in_=sim_sb[:].rearrange("p (c e) -> p c e", e=CH),
                            axis=mybir.AxisListType.X,
                            op=mybir.AluOpType.max,
                        )
                        # spill sim tile: rows q*512+chunk of width 128
                        nc.sync.dma_start(
                            sim_dram[qb][:]
                            .rearrange("(q c) e -> q c e", q=P)[:, nt * CPT : (nt + 1) * CPT, :],
                            sim_sb[:].rearrange("p (c e) -> p c e", e=CH),
                        )

            # ---------------- phases 2-5 per query block -------------------
            with (
                tc.tile_pool(name="sel_sb", bufs=2) as xp,
                tc.tile_pool(name="gat_sb", bufs=2) as gp,
                tc.tile_pool(name="mg_sb", bufs=2) as mg,
            ):
                iota_p512 = pp.tile([P, K], mybir.dt.int32)   # p*512
                nc.gpsimd.iota(iota_p512[:], pattern=[[0, K]], base=0, channel_multiplier=NCHUNK)
                iota_k32 = pp.tile([P, K], mybir.dt.int32)    # 0..31
                nc.gpsimd.iota(iota_k32[:], pattern=[[1, K]], base=0, channel_multiplier=0)
                iota_k32f = pp.tile([P, K], f32)
                nc.vector.tensor_copy(iota_k32f[:], iota_k32[:])

                for qb in range(NQB):
                    # ---- select top-32 chunks by chunk max
                    Mw = xp.tile([P, NCHUNK], f32, tag="Mw")
                    nc.vector.tensor_copy(Mw[:], M[:, qb, :])
                    cvals = xp.tile([P, K], f32, tag="cvals")
                    cidx = xp.tile([P, K], mybir.dt.uint32, tag="cidx")
                    for r in range(4):
                        nc.vector.max(out=cvals[:, r * 8 : (r + 1) * 8], in_=Mw[:])
                        nc.vector.max_index(
                            out=cidx[:, r * 8 : (r + 1) * 8],
                            in_max=cvals[:, r * 8 : (r + 1) * 8], in_values=Mw[:],
                        )
                        if r < 3:
                            nc.vector.match_replace(
                                out=Mw[:], in_to_replace=cvals[:, r * 8 : (r + 1) * 8],
                                in_values=Mw[:], imm_value=NEG,
                            )
                    cidxf = xp.tile([P, K], f32, tag="cidxf")
                    nc.vector.tensor_copy(cidxf[:], cidx[:])

                    # ---- gather the 32 chunks of sim for this block
                    goff = xp.tile([P, K], mybir.dt.int32, tag="goff")
                    nc.vector.tensor_tensor(out=goff[:], in0=iota_p512[:], in1=cidx[:].bitcast(mybir.dt.int32), op=mybir.AluOpType.add)
                    gsim = gp.tile([P, K, CH], f32, tag="gsim")
                    for j in range(K):
                        nc.gpsimd.indirect_dma_start(
                            out=gsim[:, j, :], out_offset=None,
                            in_=sim_dram[qb][:],
                            in_offset=bass.IndirectOffsetOnAxis(ap=goff[:, j : j + 1], axis=0),
                        )

                    # ---- exact top-32 of the gathered 4096 candidates
                    vals = xp.tile([P, K], f32, tag="vals")
                    pos = xp.tile([P, K], mybir.dt.uint32, tag="pos")
                    gflat = gsim[:].rearrange("p k e -> p (k e)")
                    for r in range(4):
                        nc.vector.max(out=vals[:, r * 8 : (r + 1) * 8], in_=gflat)
                        nc.vector.max_index(
                            out=pos[:, r * 8 : (r + 1) * 8],
                            in_max=vals[:, r * 8 : (r + 1) * 8], in_values=gflat,
                        )
                        if r < 3:
                            nc.vector.match_replace(
                                out=gflat, in_to_replace=vals[:, r * 8 : (r + 1) * 8],
                                in_values=gflat, imm_value=NEG,
                            )
                    nc.sync.dma_start(tsim_out[qb * P : (qb + 1) * P, :], vals[:])

                    # ---- original slot ids computed below via one-hot on pos//128
                    posf = xp.tile([P, K], f32, tag="posf")
                    nc.vector.tensor_copy(posf[:], pos[:])

                    # ---- softmax weights
                    nmx = xp.tile([P, 1], f32, tag="nmx")
                    nc.vector.tensor_scalar_mul(nmx[:], vals[:, 0:1], -1.0)
                    ex = xp.tile([P, K], f32, tag="ex")
                    nc.scalar.activation(out=ex[:], in_=vals[:], func=mybir.ActivationFunctionType.Exp, bias=nmx[:, 0:1])
                    z = xp.tile([P, 1], f32, tag="z")
                    nc.vector.tensor_reduce(out=z[:], in_=ex[:], axis=mybir.AxisListType.X, op=mybir.AluOpType.add)
                    rz = xp.tile([P, 1], f32, tag="rz")
                    nc.vector.reciprocal(rz[:], z[:])
                    w = xp.tile([P, K], f32, tag="w")
                    nc.scalar.activation(out=w[:], in_=ex[:], func=mybir.ActivationFunctionType.Copy, scale=rz[:, 0:1])

                    # ---- slot = chunkval*128 + pos - 128*c  (c via one-hot)
                    chs = xp.tile([P, K, K], f32, tag="chs")     # onehot weights
                    # onehot[p,j,c] = (floor(pos/128) == c): pos in [128c, 128c+128)
                    # compute via two comparisons on scaled iota (128*c)
                    i128 = xp.tile([P, K], f32, tag="i128")
                    nc.vector.tensor_scalar_mul(i128[:], iota_k32f[:], float(CH))
                    a = xp.tile([P, K, K], f32, tag="cmpa")
                    nc.vector.tensor_tensor(
                        out=a[:],
                        in0=posf[:].rearrange("p (j o) -> p j o", o=1).to_broadcast([P, K, K]),
                        in1=i128[:].rearrange("p (j c) -> p j c", j=1).to_broadcast([P, K, K]),
                        op=mybir.AluOpType.is_ge,
                    )
                    b = xp.tile([P, K, K], f32, tag="cmpb")
                    nc.vector.tensor_tensor(
                        out=b[:],
                        in0=posf[:].rearrange("p (j o) -> p j o", o=1).to_broadcast([P, K, K]),
                        in1=i128[:].rearrange("p (j c) -> p j c", j=1).to_broadcast([P, K, K]),
                        op=mybir.AluOpType.is_lt,
                    )
                    # shift b left by one c: onehot = a[c] * b[c+1]; b[K-1] treated as 1
                    nc.vector.tensor_tensor(out=chs[:, :, : K - 1], in0=a[:, :, : K - 1], in1=b[:, :, 1:], op=mybir.AluOpType.mult)
                    nc.vector.tensor_copy(chs[:, :, K - 1 : K], a[:, :, K - 1 : K])
                    # cf = sum_c onehot*c ; chunkv = sum_c onehot*chunkf
                    t1 = xp.tile([P, K, K], f32, tag="t1")
                    nc.vector.tensor_tensor(
                        out=t1[:], in0=chs[:],
                        in1=i128[:].rearrange("p (j c) -> p j c", j=1).to_broadcast([P, K, K]),
                        op=mybir.AluOpType.mult,
                    )
                    c128 = xp.tile([P, K], f32, tag="c128")     # 128*c
                    nc.vector.tensor_reduce(out=c128[:], in_=t1[:], axis=mybir.AxisListType.X, op=mybir.AluOpType.add)
                    nc.vector.tensor_tensor(
                        out=t1[:], in0=chs[:],
                        in1=cidxf[:].rearrange("p (j c) -> p j c", j=1).to_broadcast([P, K, K]),
                        op=mybir.AluOpType.mult,
                    )
                    chunkv = xp.tile([P, K], f32, tag="chunkv")
                    nc.vector.tensor_reduce(out=chunkv[:], in_=t1[:], axis=mybir.AxisListType.X, op=mybir.AluOpType.add)
                    slot = xp.tile([P, K], f32, tag="slot")
                    nc.vector.tensor_scalar_mul(slot[:], chunkv[:], float(CH))
                    nc.vector.tensor_add(slot[:], slot[:], posf[:])
                    nc.vector.tensor_sub(slot[:], slot[:], c128[:])
                    sloti = xp.tile([P, K], mybir.dt.int32, tag="sloti")
                    nc.vector.tensor_copy(sloti[:], slot[:])

                    # ---- gather memory rows and weighted-sum
                    retr = mg.tile([P, D], f32, tag="retr")
                    nc.vector.memset(retr[:], 0.0)
                    for jg in range(K // 4):
                        g = mg.tile([P, 4, D], f32, tag="g")
                        for j in range(4):
                            nc.gpsimd.indirect_dma_start(
                                out=g[:, j, :], out_offset=None,
                                in_=mem[:],
                                in_offset=bass.IndirectOffsetOnAxis(ap=sloti[:, jg * 4 + j : jg * 4 + j + 1], axis=0),
                            )
                        for j in range(4):
                            nc.scalar.activation(
                                out=g[:, j, :], in_=g[:, j, :],
                                func=mybir.ActivationFunctionType.Copy,
                                scale=w[:, jg * 4 + j : jg * 4 + j + 1],
                            )
                        part = mg.tile([P, D], f32, tag="part")
                        nc.vector.tensor_reduce(
                            out=part[:], in_=g[:].rearrange("p j d -> p d j"),
                            axis=mybir.AxisListType.X, op=mybir.AluOpType.add,
                        )
                        nc.vector.tensor_add(retr[:], retr[:], part[:])
                    nc.sync.dma_start(retr_out[qb * P : (qb + 1) * P, :], retr[:])

    nc.finalize()
    _split_waits(nc, limit=1)
    return nc


def kernel(query, memory, W_key, top_k):
    assert int(top_k) == K
    query = np.ascontiguousarray(np.asarray(query, dtype=np.float32))
    memory = np.ascontiguousarray(np.asarray(memory, dtype=np.float32))
    W_key = np.asarray(W_key, dtype=np.float32)
    assert query.shape == (NCORES * BC, D) and memory.shape == (N, D)

    if "nc" not in _cache:
        _cache["nc"] = _build()
    nc = _cache["nc"]

    WT = np.ascontiguousarray(W_key.T)
    memT = np.ascontiguousarray(memory.T)
    in_maps = []
    for c in range(NCORES):
        qT = np.ascontiguousarray(query[c * BC : (c + 1) * BC].T)
        in_maps.append({"qT": qT, "WT": WT, "memT": memT, "mem": memory})

    import os

    trace = os.environ.get("BASS_TRACE_KERNEL") == "1"
    res = bass_utils.run_bass_kernel_spmd(
        nc, in_maps, core_ids=list(range(NCORES)), trace=trace
    )
    _cache["last_results"] = res
    retrieved = np.concatenate([res.results[c]["retrieved"] for c in range(NCORES)], axis=0)
    top_sim = np.concatenate([res.results[c]["top_sim"] for c in range(NCORES)], axis=0)
    return retrieved, top_sim


# revision 14
# speedup vs baseline: 1.2610x; 1.2610x over previous
"""BTSP memory-bank retrieval kernel for 8 Trainium2 NeuronCores.

Problem: query (4096,1024) f32, memory (65536,1024) f32 unit-norm rows,
W_key (1024,1024) f32, top_k=32.
  query_proj = query @ W_key.T ; qn = l2norm(query_proj)
  sim = qn @ memory.T ; top_sim, top_idx = top_k(sim, 32)
  weights = softmax(top_sim) ; retrieved = weights . memory[top_idx]
Returns (retrieved (4096,1024) f32, top_sim (4096,32) f32).

Sharding: pure data-parallel over queries (B) — each of the 8 cores handles
512 queries against the full memory bank; no collectives. Inside one core:
  - fp32 matmul (PE) computes sim in 512-column tiles, streamed over N
  - per-row chunk maxima (chunks of 128) feed a top-32-chunks tournament:
    the 32nd-largest chunk-max is a valid threshold (32 distinct elements
    are >= it), so the true top-32 elements all live in the 32 selected
    chunks; gather those 4096 candidates and take an exact top-32 with the
    DVE max8/max_index/match_replace instructions
  - softmax over the 32 values, indirect-DMA gather of the 32 memory rows,
    weighted sum -> retrieved.
"""
import numpy as np
import concourse.bass as bass
import concourse.mybir as mybir
from concourse.tile import TileContext
from concourse import bass_utils
from concourse.masks import make_identity

P = 128          # partitions
D = 1024         # feature dim
N = 65536        # memory slots
K = 32           # top-k
BC = 512         # queries per core
NQB = BC // P    # query blocks per core (4)
NT = 512         # sim tile width (N columns per PSUM tile)
NTILES = N // NT # 128
CH = 128         # chunk width for chunk-max tournament
CPT = NT // CH   # chunks per sim tile (4)
NCHUNK = N // CH # chunks per row (512)
NCORES = 8
NEG = -1e30

_cache = {}


def _split_waits(nc, limit=1):
    """Walrus in this container accepts at most `limit` semaphore waits per
    instruction; move excess waits onto preceding same-engine NOPs."""
    n_split = 0
    for f in nc.m.functions:
        for bb in f.blocks:
            new_list = []
            changed = False
            for ins in bb.instructions:
                si = getattr(ins, "sync_info", None)
                waits = list(si.on_wait) if (si is not None and si.on_wait) else []
                if len(waits) > limit:
                    changed = True
                    n_split += 1
                    extra, keep = waits[:-limit], waits[-limit:]
                    for j in range(0, len(extra), limit):
                        nop = mybir.InstNoOp(
                            name=nc.get_next_instruction_name(), ins=[], outs=[]
                        )
                        nop.engine = ins.engine
                        nop.sync_info = mybir.SyncInfo(
                            on_wait=extra[j : j + limit], on_update=[]
                        )
                        new_list.append(nop)
                    si.on_wait = keep
                new_list.append(ins)
            if changed:
                bb.instructions[:] = new_list
    return n_split


def _build():
    f32 = mybir.dt.float32
    nc = bass.Bass(trn_type="TRN2")

    f32r = mybir.dt.float32r
    qT = nc.dram_tensor("qT", [D, BC], f32, kind="ExternalInput")        # query shard, transposed
    WT = nc.dram_tensor("WT", [D, D], f32, kind="ExternalInput")         # W_key.T ([din, dout])
    # memory transposed, split on host into 12-bit-mantissa value + residual
    # (the PE's fp32r format): memT ~= memTr + memTres exactly to ~2^-24.
    memTr = nc.dram_tensor("memTr", [D, N], f32r, kind="ExternalInput")
    memTres = nc.dram_tensor("memTres", [D, N], f32r, kind="ExternalInput")
    mem = nc.dram_tensor("mem", [N, D], f32, kind="ExternalInput")       # memory (row gather)

    retr_out = nc.dram_tensor("retrieved", [BC, D], f32, kind="ExternalOutput")
    tsim_out = nc.dram_tensor("top_sim", [BC, K], f32, kind="ExternalOutput")

    with TileContext(nc) as tc:
        with (
            tc.tile_pool(name="persist", bufs=1) as pp,
            tc.tile_pool(name="dram", bufs=1, space="DRAM") as dp,
        ):
            # ---------------- phase 0: query proj + l2norm + transpose -----
            qnT_r = pp.tile([P, NQB, 8, P], f32r)    # R12(qnT)
            qnT_rr = pp.tile([P, NQB, 8, P], f32r)   # R12(qnT - R12(qnT))
            ident = pp.tile([P, P], f32)
            make_identity(nc, ident[:])
            M = pp.tile([P, NQB, NCHUNK], f32)     # chunk maxima per query block

            with (
                tc.tile_pool(name="proj_sb", bufs=2) as sb0,
                tc.tile_pool(name="proj_ps", bufs=2, space="PSUM") as ps0,
                tc.tile_pool(name="tr_ps", bufs=2, space="PSUM") as ps0t,
            ):
                qt_all = sb0.tile([P, 8, NQB, P], f32, tag="qt")   # [din128, k, qb, q]
                nc.sync.dma_start(
                    qt_all[:], qT[:].rearrange("(k p) (qb q) -> p k qb q", p=P, q=P)
                )
                wt_all = sb0.tile([P, 8, D], f32, tag="wt")        # [din128, k, dout]
                nc.sync.dma_start(wt_all[:], WT[:].rearrange("(k p) d -> p k d", p=P))

                for qb in range(NQB):
                    qp = sb0.tile([P, D], f32, tag="qp")           # [q, dout]
                    for half in range(2):
                        psq = ps0.tile([P, NT], f32, space="PSUM", tag="psq")
                        for k in range(8):
                            nc.tensor.matmul(
                                psq[:],
                                qt_all[:, k, qb, :],
                                wt_all[:, k, half * NT : (half + 1) * NT],
                                start=(k == 0),
                                stop=(k == 7),
                            )
                        nc.vector.tensor_copy(qp[:, half * NT : (half + 1) * NT], psq[:])
                    sq = sb0.tile([P, D], f32, tag="sq")
                    nc.vector.tensor_tensor(out=sq[:], in0=qp[:], in1=qp[:], op=mybir.AluOpType.mult)
                    nrm2 = sb0.tile([P, 1], f32, tag="n2")
                    nc.vector.tensor_reduce(out=nrm2[:], in_=sq[:], axis=mybir.AxisListType.X, op=mybir.AluOpType.add)
                    nrm = sb0.tile([P, 1], f32, tag="nr")
                    nc.scalar.activation(out=nrm[:], in_=nrm2[:], func=mybir.ActivationFunctionType.Sqrt)
                    nc.vector.tensor_scalar_max(nrm[:], nrm[:], 1e-12)
                    rn = sb0.tile([P, 1], f32, tag="rn")
                    nc.vector.reciprocal(rn[:], nrm[:])
                    qn = sb0.tile([P, D], f32, tag="qn")
                    nc.scalar.activation(out=qn[:], in_=qp[:], func=mybir.ActivationFunctionType.Copy, scale=rn[:, 0:1])
                    # transpose 128x128 blocks, then split into fp32r value+residual
                    for k in range(8):
                        pst = ps0t.tile([P, P], f32, space="PSUM", tag="pst")
                        nc.tensor.transpose(pst[:], qn[:, k * P : (k + 1) * P], ident[:])
                        qnt_f = sb0.tile([P, P], f32, tag="qntf")
                        nc.vector.tensor_copy(qnt_f[:], pst[:])
                        nc.vector.tensor_copy(qnT_r[:, qb, k, :], qnt_f[:])
                        resid = sb0.tile([P, P], f32, tag="resid")
                        nc.vector.tensor_sub(
                            resid[:], qnt_f[:], qnT_r[:, qb, k, :].bitcast(f32)
                        )
                        nc.vector.tensor_copy(qnT_rr[:, qb, k, :], resid[:])

            # ---------------- phase 1: sim matmul + chunk maxima -----------
            # one DRAM scratch per query block: indirect-DMA sources need offset 0
            sim_dram = [
                dp.tile([P * NCHUNK, CH], f32, name=f"sim_scratch{i}", tag=f"sim_scratch{i}")
                for i in range(NQB)
            ]
            with (
                tc.tile_pool(name="mem_sb", bufs=2) as mp,
                tc.tile_pool(name="sim_sb", bufs=4) as sp,
                tc.tile_pool(name="sim_ps", bufs=2, space="PSUM") as psp,
            ):
                for nt in range(NTILES):
                    mtr = mp.tile([P, 8, NT], f32r, tag="mtr")
                    nc.sync.dma_start(
                        mtr[:], memTr[:, nt * NT : (nt + 1) * NT].rearrange("(k p) n -> p k n", p=P)
                    )
                    mtres = mp.tile([P, 8, NT], f32r, tag="mtres")
                    nc.sync.dma_start(
                        mtres[:], memTres[:, nt * NT : (nt + 1) * NT].rearrange("(k p) n -> p k n", p=P)
                    )
                    for qb in range(NQB):
                        ps = psp.tile([P, NT], f32, space="PSUM", tag=f"ps{qb}")
                        # sim = R(q)R(m) + R(q_res)R(m) + R(q)R(m_res)
                        for k in range(8):
                            nc.tensor.matmul(
                                ps[:], qnT_r[:, qb, k, :], mtr[:, k, :],
                                start=(k == 0), stop=False,
                            )
                        for k in range(8):
                            nc.tensor.matmul(
                                ps[:], qnT_rr[:, qb, k, :], mtr[:, k, :],
                                start=False, stop=False,
                            )
                        for k in range(8):
                            nc.tensor.matmul(
                                ps[:], qnT_r[:, qb, k, :], mtres[:, k, :],
                                start=False, stop=(k == 7),
                            )
                        sim_sb = sp.tile([P, NT], f32, tag="sim")
                        nc.vector.tensor_copy(sim_sb[:], ps[:])
                        # chunk maxima -> M
                        nc.vector.tensor_reduce(
                            out=M[:, qb, nt * CPT : (nt + 1) * CPT],
                            in_=sim_sb[:].rearrange("p (c e) -> p c e", e=CH),
                            axis=mybir.AxisListType.X,
                            op=mybir.AluOpType.max,
                        )
                        # spill sim tile: rows q*512+chunk of width 128
                        nc.sync.dma_start(
                            sim_dram[qb][:]
                            .rearrange("(q c) e -> q c e", q=P)[:, nt * CPT : (nt + 1) * CPT, :],
                            sim_sb[:].rearrange("p (c e) -> p c e", e=CH),
                        )

            # ---------------- phases 2-5 per query block -------------------
            with (
                tc.tile_pool(name="sel_sb", bufs=2) as xp,
                tc.tile_pool(name="gat_sb", bufs=1) as gp,
                tc.tile_pool(name="mg_sb", bufs=1) as mg,
            ):
                iota_p512 = pp.tile([P, K], mybir.dt.int32)   # p*512
                nc.gpsimd.iota(iota_p512[:], pattern=[[0, K]], base=0, channel_multiplier=NCHUNK)
                iota_k32 = pp.tile([P, K], mybir.dt.int32)    # 0..31
                nc.gpsimd.iota(iota_k32[:], pattern=[[1, K]], base=0, channel_multiplier=0)
                iota_k32f = pp.tile([P, K], f32)
                nc.vector.tensor_copy(iota_k32f[:], iota_k32[:])

                for qb in range(NQB):
                    # ---- select top-32 chunks by chunk max
                    Mw = xp.tile([P, NCHUNK], f32, tag="Mw")
                    nc.vector.tensor_copy(Mw[:], M[:, qb, :])
                    cvals = xp.tile([P, K], f32, tag="cvals")
                    cidx = xp.tile([P, K], mybir.dt.uint32, tag="cidx")
                    for r in range(4):
                        nc.vector.max(out=cvals[:, r * 8 : (r + 1) * 8], in_=Mw[:])
                        nc.vector.max_index(
                            out=cidx[:, r * 8 : (r + 1) * 8],
                            in_max=cvals[:, r * 8 : (r + 1) * 8], in_values=Mw[:],
                        )
                        if r < 3:
                            nc.vector.match_replace(
                                out=Mw[:], in_to_replace=cvals[:, r * 8 : (r + 1) * 8],
                                in_values=Mw[:], imm_value=NEG,
                            )
                    cidxf = xp.tile([P, K], f32, tag="cidxf")
                    nc.vector.tensor_copy(cidxf[:], cidx[:])

                    # ---- gather the 32 chunks of sim for this block
                    goff = xp.tile([P, K], mybir.dt.int32, tag="goff")
                    nc.vector.tensor_tensor(out=goff[:], in0=iota_p512[:], in1=cidx[:].bitcast(mybir.dt.int32), op=mybir.AluOpType.add)
                    gsim = gp.tile([P, K, CH], f32, tag="gsim")
                    for j in range(K):
                        nc.gpsimd.indirect_dma_start(
                            out=gsim[:, j, :], out_offset=None,
                            in_=sim_dram[qb][:],
                            in_offset=bass.IndirectOffsetOnAxis(ap=goff[:, j : j + 1], axis=0),
                        )

                    # ---- exact top-32 of the gathered 4096 candidates
                    vals = xp.tile([P, K], f32, tag="vals")
                    pos = xp.tile([P, K], mybir.dt.uint32, tag="pos")
                    gflat = gsim[:].rearrange("p k e -> p (k e)")
                    for r in range(4):
                        nc.vector.max(out=vals[:, r * 8 : (r + 1) * 8], in_=gflat)
                        nc.vector.max_index(
                            out=pos[:, r * 8 : (r + 1) * 8],
                            in_max=vals[:, r * 8 : (r + 1) * 8], in_values=gflat,
                        )
                        if r < 3:
                            nc.vector.match_replace(
                                out=gflat, in_to_replace=vals[:, r * 8 : (r + 1) * 8],
                                in_values=gflat, imm_value=NEG,
                            )
                    nc.sync.dma_start(tsim_out[qb * P : (qb + 1) * P, :], vals[:])

                    # ---- original slot ids computed below via one-hot on pos//128
                    posf = xp.tile([P, K], f32, tag="posf")
                    nc.vector.tensor_copy(posf[:], pos[:])

                    # ---- softmax weights
                    nmx = xp.tile([P, 1], f32, tag="nmx")
                    nc.vector.tensor_scalar_mul(nmx[:], vals[:, 0:1], -1.0)
                    ex = xp.tile([P, K], f32, tag="ex")
                    nc.scalar.activation(out=ex[:], in_=vals[:], func=mybir.ActivationFunctionType.Exp, bias=nmx[:, 0:1])
                    z = xp.tile([P, 1], f32, tag="z")
                    nc.vector.tensor_reduce(out=z[:], in_=ex[:], axis=mybir.AxisListType.X, op=mybir.AluOpType.add)
                    rz = xp.tile([P, 1], f32, tag="rz")
                    nc.vector.reciprocal(rz[:], z[:])
                    w = xp.tile([P, K], f32, tag="w")
                    nc.scalar.activation(out=w[:], in_=ex[:], func=mybir.ActivationFunctionType.Copy, scale=rz[:, 0:1])

                    # ---- slot = chunkval*128 + pos - 128*c  (c via one-hot)
                    chs = xp.tile([P, K, K], f32, tag="chs")     # onehot weights
                    # onehot[p,j,c] = (floor(pos/128) == c): pos in [128c, 128c+128)
                    # compute via two comparisons on scaled iota (128*c)
                    i128 = xp.tile([P, K], f32, tag="i128")
                    nc.vector.tensor_scalar_mul(i128[:], iota_k32f[:], float(CH))
                    a = xp.tile([P, K, K], f32, tag="cmpa")
                    nc.vector.tensor_tensor(
                        out=a[:],
                        in0=posf[:].rearrange("p (j o) -> p j o", o=1).to_broadcast([P, K, K]),
                        in1=i128[:].rearrange("p (j c) -> p j c", j=1).to_broadcast([P, K, K]),
                        op=mybir.AluOpType.is_ge,
                    )
                    b = xp.tile([P, K, K], f32, tag="cmpb")
                    nc.vector.tensor_tensor(
                        out=b[:],
                        in0=posf[:].rearrange("p (j o) -> p j o", o=1).to_broadcast([P, K, K]),
                        in1=i128[:].rearrange("p (j c) -> p j c", j=1).to_broadcast([P, K, K]),
                        op=mybir.AluOpType.is_lt,
                    )
                    # shift b left by one c: onehot = a[c] * b[c+1]; b[K-1] treated as 1
                    nc.vector.tensor_tensor(out=chs[:, :, : K - 1], in0=a[:, :, : K - 1], in1=b[:, :, 1:], op=mybir.AluOpType.mult)
                    nc.vector.tensor_copy(chs[:, :, K - 1 : K], a[:, :, K - 1 : K])
                    # cf = sum_c onehot*c ; chunkv = sum_c onehot*chunkf
                    t1 = xp.tile([P, K, K], f32, tag="t1")
                    nc.vector.tensor_tensor(
                        out=t1[:], in0=chs[:],
                        in1=i128[:].rearrange("p (j c) -> p j c", j=1).to_broadcast([P, K, K]),
                        op=mybir.AluOpType.mult,
                    )
                    c128 = xp.tile([P, K], f32, tag="c128")     # 128*c
                    nc.vector.tensor_reduce(out=c128[:], in_=t1[:], axis=mybir.AxisListType.X, op=mybir.AluOpType.add)
                    nc.vector.tensor_tensor(
                        out=t1[:], in0=chs[:],
                        in1=cidxf[:].rearrange("p (j c) -> p j c", j=1).to_broadcast([P, K, K]),
                        op=mybir.AluOpType.mult,
                    )
                    chunkv = xp.tile([P, K], f32, tag="chunkv")
                    nc.vector.tensor_reduce(out=chunkv[:], in_=t1[:], axis=mybir.AxisListType.X, op=mybir.AluOpType.add)
                    slot = xp.tile([P, K], f32, tag="slot")
                    nc.vector.tensor_scalar_mul(slot[:], chunkv[:], float(CH))
                    nc.vector.tensor_add(slot[:], slot[:], posf[:])
                    nc.vector.tensor_sub(slot[:], slot[:], c128[:])
                    sloti = xp.tile([P, K], mybir.dt.int32, tag="sloti")
                    nc.vector.tensor_copy(sloti[:], slot[:])

                    # ---- gather memory rows and weighted-sum
                    retr = mg.tile([P, D], f32, tag="retr")
                    nc.vector.memset(retr[:], 0.0)
                    for jg in range(K // 4):
                        g = mg.tile([P, 4, D], f32, tag="g")
                        for j in range(4):
                            nc.gpsimd.indirect_dma_start(
                                out=g[:, j, :], out_offset=None,
                                in_=mem[:],
                                in_offset=bass.IndirectOffsetOnAxis(ap=sloti[:, jg * 4 + j : jg * 4 + j + 1], axis=0),
                            )
                        for j in range(4):
                            nc.scalar.activation(
                                out=g[:, j, :], in_=g[:, j, :],
                                func=mybir.ActivationFunctionType.Copy,
                                scale=w[:, jg * 4 + j : jg * 4 + j + 1],
                            )
                        part = mg.tile([P, D], f32, tag="part")
                        nc.vector.tensor_reduce(
                            out=part[:], in_=g[:].rearrange("p j d -> p d j"),
                            axis=mybir.AxisListType.X, op=mybir.AluOpType.add,
                        )
                        nc.vector.tensor_add(retr[:], retr[:], part[:])
                    nc.sync.dma_start(retr_out[qb * P : (qb + 1) * P, :], retr[:])

    nc.finalize()
    _split_waits(nc, limit=1)
    return nc


def kernel(query, memory, W_key, top_k):
    assert int(top_k) == K
    query = np.ascontiguousarray(np.asarray(query, dtype=np.float32))
    memory = np.ascontiguousarray(np.asarray(memory, dtype=np.float32))
    W_key = np.asarray(W_key, dtype=np.float32)
    assert query.shape == (NCORES * BC, D) and memory.shape == (N, D)

    if "nc" not in _cache:
        _cache["nc"] = _build()
    nc = _cache["nc"]

    WT = np.ascontiguousarray(W_key.T)
    memT = np.ascontiguousarray(memory.T)

    def round12(x):
        # the PE's fp32r input format: round-to-nearest, 12-bit mantissa
        mant, exp = np.frexp(x)
        return np.ldexp(np.round(mant * 4096.0) / np.float32(4096.0), exp).astype(np.float32)

    memTr = round12(memT)
    memTres = round12(memT - memTr)
    in_maps = []
    for c in range(NCORES):
        qT = np.ascontiguousarray(query[c * BC : (c + 1) * BC].T)
        in_maps.append({"qT": qT, "WT": WT, "memTr": memTr, "memTres": memTres, "mem": memory})

    import os

    trace = os.environ.get("BASS_TRACE_KERNEL") == "1"
    res = bass_utils.run_bass_kernel_spmd(
        nc, in_maps, core_ids=list(range(NCORES)), trace=trace
    )
    _cache["last_results"] = res
    retrieved = np.concatenate([res.results[c]["retrieved"] for c in range(NCORES)], axis=0)
    top_sim = np.concatenate([res.results[c]["top_sim"] for c in range(NCORES)], axis=0)
    return retrieved, top_sim


# revision 18
# speedup vs baseline: 1.3139x; 1.0420x over previous
"""BTSP memory-bank retrieval kernel for 8 Trainium2 NeuronCores.

Problem: query (4096,1024) f32, memory (65536,1024) f32 unit-norm rows,
W_key (1024,1024) f32, top_k=32.
  query_proj = query @ W_key.T ; qn = l2norm(query_proj)
  sim = qn @ memory.T ; top_sim, top_idx = top_k(sim, 32)
  weights = softmax(top_sim) ; retrieved = weights . memory[top_idx]
Returns (retrieved (4096,1024) f32, top_sim (4096,32) f32).

Sharding: pure data-parallel over queries (B) — each of the 8 cores handles
512 queries against the full memory bank; no collectives. Inside one core:
  - fp32 matmul (PE) computes sim in 512-column tiles, streamed over N
  - per-row chunk maxima (chunks of 128) feed a top-32-chunks tournament:
    the 32nd-largest chunk-max is a valid threshold (32 distinct elements
    are >= it), so the true top-32 elements all live in the 32 selected
    chunks; gather those 4096 candidates and take an exact top-32 with the
    DVE max8/max_index/match_replace instructions
  - softmax over the 32 values, indirect-DMA gather of the 32 memory rows,
    weighted sum -> retrieved.
"""
import numpy as np
import concourse.bass as bass
import concourse.mybir as mybir
from concourse.tile import TileContext
from concourse import bass_utils
from concourse.masks import make_identity

P = 128          # partitions
D = 1024         # feature dim
N = 65536        # memory slots
K = 32           # top-k
BC = 512         # queries per core
NQB = BC // P    # query blocks per core (4)
NT = 512         # sim tile width (N columns per PSUM tile)
NTILES = N // NT # 128
CH = 64          # chunk width for chunk-max tournament
CPT = NT // CH   # chunks per sim tile (8)
NCHUNK = N // CH # chunks per row (1024)
NCORES = 8
NEG = -1e30

_cache = {}


def _split_waits(nc, limit=1):
    """Walrus in this container accepts at most `limit` semaphore waits per
    instruction; move excess waits onto preceding same-engine NOPs."""
    n_split = 0
    for f in nc.m.functions:
        for bb in f.blocks:
            new_list = []
            changed = False
            for ins in bb.instructions:
                si = getattr(ins, "sync_info", None)
                waits = list(si.on_wait) if (si is not None and si.on_wait) else []
                if len(waits) > limit:
                    changed = True
                    n_split += 1
                    extra, keep = waits[:-limit], waits[-limit:]
                    for j in range(0, len(extra), limit):
                        nop = mybir.InstNoOp(
                            name=nc.get_next_instruction_name(), ins=[], outs=[]
                        )
                        nop.engine = ins.engine
                        nop.sync_info = mybir.SyncInfo(
                            on_wait=extra[j : j + limit], on_update=[]
                        )
                        new_list.append(nop)
                    si.on_wait = keep
                new_list.append(ins)
            if changed:
                bb.instructions[:] = new_list
    return n_split


def _build():
    f32 = mybir.dt.float32
    nc = bass.Bass(trn_type="TRN2")

    f32r = mybir.dt.float32r
    qT = nc.dram_tensor("qT", [D, BC], f32, kind="ExternalInput")        # query shard, transposed
    WT = nc.dram_tensor("WT", [D, D], f32, kind="ExternalInput")         # W_key.T ([din, dout])
    # memory transposed, split on host into 12-bit-mantissa value + residual
    # (the PE's fp32r format): memT ~= memTr + memTres exactly to ~2^-24.
    memTr = nc.dram_tensor("memTr", [D, N], f32r, kind="ExternalInput")
    memTres = nc.dram_tensor("memTres", [D, N], f32r, kind="ExternalInput")
    mem = nc.dram_tensor("mem", [N, D], f32, kind="ExternalInput")       # memory (row gather)

    retr_out = nc.dram_tensor("retrieved", [BC, D], f32, kind="ExternalOutput")
    tsim_out = nc.dram_tensor("top_sim", [BC, K], f32, kind="ExternalOutput")

    with TileContext(nc) as tc:
        with (
            tc.tile_pool(name="persist", bufs=1) as pp,
            tc.tile_pool(name="dram", bufs=1, space="DRAM") as dp,
        ):
            # ---------------- phase 0: query proj + l2norm + transpose -----
            qnT_r = pp.tile([P, NQB, 8, P], f32r)    # R12(qnT)
            qnT_rr = pp.tile([P, NQB, 8, P], f32r)   # R12(qnT - R12(qnT))
            ident = pp.tile([P, P], f32)
            make_identity(nc, ident[:])
            M = pp.tile([P, NQB, NCHUNK], f32)     # chunk maxima per query block

            with (
                tc.tile_pool(name="proj_sb", bufs=2) as sb0,
                tc.tile_pool(name="proj_ps", bufs=2, space="PSUM") as ps0,
                tc.tile_pool(name="tr_ps", bufs=2, space="PSUM") as ps0t,
            ):
                qt_all = sb0.tile([P, 8, NQB, P], f32, tag="qt")   # [din128, k, qb, q]
                nc.sync.dma_start(
                    qt_all[:], qT[:].rearrange("(k p) (qb q) -> p k qb q", p=P, q=P)
                )
                wt_all = sb0.tile([P, 8, D], f32, tag="wt")        # [din128, k, dout]
                nc.sync.dma_start(wt_all[:], WT[:].rearrange("(k p) d -> p k d", p=P))

                for qb in range(NQB):
                    qp = sb0.tile([P, D], f32, tag="qp")           # [q, dout]
                    for half in range(2):
                        psq = ps0.tile([P, NT], f32, space="PSUM", tag="psq")
                        for k in range(8):
                            nc.tensor.matmul(
                                psq[:],
                                qt_all[:, k, qb, :],
                                wt_all[:, k, half * NT : (half + 1) * NT],
                                start=(k == 0),
                                stop=(k == 7),
                            )
                        nc.vector.tensor_copy(qp[:, half * NT : (half + 1) * NT], psq[:])
                    sq = sb0.tile([P, D], f32, tag="sq")
                    nc.vector.tensor_tensor(out=sq[:], in0=qp[:], in1=qp[:], op=mybir.AluOpType.mult)
                    nrm2 = sb0.tile([P, 1], f32, tag="n2")
                    nc.vector.tensor_reduce(out=nrm2[:], in_=sq[:], axis=mybir.AxisListType.X, op=mybir.AluOpType.add)
                    nrm = sb0.tile([P, 1], f32, tag="nr")
                    nc.scalar.activation(out=nrm[:], in_=nrm2[:], func=mybir.ActivationFunctionType.Sqrt)
                    nc.vector.tensor_scalar_max(nrm[:], nrm[:], 1e-12)
                    rn = sb0.tile([P, 1], f32, tag="rn")
                    nc.vector.reciprocal(rn[:], nrm[:])
                    qn = sb0.tile([P, D], f32, tag="qn")
                    nc.scalar.activation(out=qn[:], in_=qp[:], func=mybir.ActivationFunctionType.Copy, scale=rn[:, 0:1])
                    # transpose 128x128 blocks, then split into fp32r value+residual
                    for k in range(8):
                        pst = ps0t.tile([P, P], f32, space="PSUM", tag="pst")
                        nc.tensor.transpose(pst[:], qn[:, k * P : (k + 1) * P], ident[:])
                        qnt_f = sb0.tile([P, P], f32, tag="qntf")
                        nc.vector.tensor_copy(qnt_f[:], pst[:])
                        nc.vector.tensor_copy(qnT_r[:, qb, k, :], qnt_f[:])
                        resid = sb0.tile([P, P], f32, tag="resid")
                        nc.vector.tensor_sub(
                            resid[:], qnt_f[:], qnT_r[:, qb, k, :].bitcast(f32)
                        )
                        nc.vector.tensor_copy(qnT_rr[:, qb, k, :], resid[:])

            # ---------------- phase 1: sim matmul + chunk maxima -----------
            # one DRAM scratch per query block: indirect-DMA sources need offset 0
            sim_dram = [
                dp.tile([P * NCHUNK, CH], f32, name=f"sim_scratch{i}", tag=f"sim_scratch{i}")
                for i in range(NQB)
            ]
            with (
                tc.tile_pool(name="mem_sb", bufs=2) as mp,
                tc.tile_pool(name="sim_sb", bufs=4) as sp,
                tc.tile_pool(name="sim_ps", bufs=2, space="PSUM") as psp,
            ):
                for nt in range(NTILES):
                    mtr = mp.tile([P, 8, NT], f32r, tag="mtr")
                    nc.sync.dma_start(
                        mtr[:], memTr[:, nt * NT : (nt + 1) * NT].rearrange("(k p) n -> p k n", p=P)
                    )
                    mtres = mp.tile([P, 8, NT], f32r, tag="mtres")
                    nc.sync.dma_start(
                        mtres[:], memTres[:, nt * NT : (nt + 1) * NT].rearrange("(k p) n -> p k n", p=P)
                    )
                    for qb in range(NQB):
                        ps = psp.tile([P, NT], f32, space="PSUM", tag=f"ps{qb}")
                        # sim = R(q)R(m) + R(q)R(m_res) + R(q_res)R(m)
                        # (same-weight matmuls paired for weight-load reuse)
                        for k in range(8):
                            nc.tensor.matmul(
                                ps[:], qnT_r[:, qb, k, :], mtr[:, k, :],
                                start=(k == 0), stop=False,
                            )
                            nc.tensor.matmul(
                                ps[:], qnT_r[:, qb, k, :], mtres[:, k, :],
                                start=False, stop=False,
                            )
                        for k in range(8):
                            nc.tensor.matmul(
                                ps[:], qnT_rr[:, qb, k, :], mtr[:, k, :],
                                start=False, stop=(k == 7),
                            )
                        sim_sb = sp.tile([P, NT], f32, tag="sim")
                        nc.vector.tensor_copy(sim_sb[:], ps[:])
                        # chunk maxima -> M
                        nc.vector.tensor_reduce(
                            out=M[:, qb, nt * CPT : (nt + 1) * CPT],
                            in_=sim_sb[:].rearrange("p (c e) -> p c e", e=CH),
                            axis=mybir.AxisListType.X,
                            op=mybir.AluOpType.max,
                        )
                        # spill sim tile: rows q*512+chunk of width 128
                        nc.sync.dma_start(
                            sim_dram[qb][:]
                            .rearrange("(q c) e -> q c e", q=P)[:, nt * CPT : (nt + 1) * CPT, :],
                            sim_sb[:].rearrange("p (c e) -> p c e", e=CH),
                        )

            # ---------------- phases 2-5: chunk select, gather, exact top-k ------
            with (
                tc.tile_pool(name="sel_sb", bufs=2) as xp,
                tc.tile_pool(name="sml_sb", bufs=1) as sm,
                tc.tile_pool(name="gat_sb", bufs=2) as gp,
                tc.tile_pool(name="mg_sb", bufs=2) as mg,
            ):
                iota_pN = pp.tile([P, K], mybir.dt.int32)     # p*NCHUNK
                nc.gpsimd.iota(iota_pN[:], pattern=[[0, K]], base=0, channel_multiplier=NCHUNK)
                iota_k32 = pp.tile([P, K], mybir.dt.int32)    # 0..31
                nc.gpsimd.iota(iota_k32[:], pattern=[[1, K]], base=0, channel_multiplier=0)
                iota_k32f = pp.tile([P, K], f32)
                nc.vector.tensor_copy(iota_k32f[:], iota_k32[:])
                iCH = pp.tile([P, K], f32)                    # CH * (0..31)
                nc.vector.tensor_scalar_mul(iCH[:], iota_k32f[:], float(CH))

                cidxf_t, goff_t = {}, {}
                # stage A: top-32 chunks per block (DVE) + gather offsets
                for qb in range(NQB):
                    Mw = xp.tile([P, NCHUNK], f32, tag="Mw")
                    nc.vector.tensor_copy(Mw[:], M[:, qb, :])
                    cvals = xp.tile([P, K], f32, tag="cvals")
                    cidx = xp.tile([P, K], mybir.dt.uint32, tag="cidx")
                    for r in range(4):
                        nc.vector.max(out=cvals[:, r * 8 : (r + 1) * 8], in_=Mw[:])
                        nc.vector.max_index(
                            out=cidx[:, r * 8 : (r + 1) * 8],
                            in_max=cvals[:, r * 8 : (r + 1) * 8], in_values=Mw[:],
                        )
                        if r < 3:
                            nc.vector.match_replace(
                                out=Mw[:], in_to_replace=cvals[:, r * 8 : (r + 1) * 8],
                                in_values=Mw[:], imm_value=NEG,
                            )
                    cidxf = sm.tile([P, K], f32, tag=f"cidxf{qb}")
                    nc.vector.tensor_copy(cidxf[:], cidx[:])
                    cidxf_t[qb] = cidxf
                    goff = sm.tile([P, K], mybir.dt.int32, tag=f"goff{qb}")
                    nc.vector.tensor_tensor(
                        out=goff[:], in0=iota_pN[:],
                        in1=cidx[:].bitcast(mybir.dt.int32), op=mybir.AluOpType.add,
                    )
                    goff_t[qb] = goff

                # stages B-D per block, pipelined via bufs=2 pools
                for qb in range(NQB):
                    gsim = gp.tile([P, K, CH], f32, tag="gsim")
                    for j in range(K):
                        nc.gpsimd.indirect_dma_start(
                            out=gsim[:, j, :], out_offset=None,
                            in_=sim_dram[qb][:],
                            in_offset=bass.IndirectOffsetOnAxis(ap=goff_t[qb][:, j : j + 1], axis=0),
                        )

                    gflat = gsim[:].rearrange("p k e -> p (k e)")
                    vals = sm.tile([P, K], f32, tag=f"vals{qb}")
                    pos = xp.tile([P, K], mybir.dt.uint32, tag="pos")
                    for r in range(4):
                        nc.vector.max(out=vals[:, r * 8 : (r + 1) * 8], in_=gflat)
                        nc.vector.max_index(
                            out=pos[:, r * 8 : (r + 1) * 8],
                            in_max=vals[:, r * 8 : (r + 1) * 8], in_values=gflat,
                        )
                        if r < 3:
                            nc.vector.match_replace(
                                out=gflat, in_to_replace=vals[:, r * 8 : (r + 1) * 8],
                                in_values=gflat, imm_value=NEG,
                            )
                    nc.sync.dma_start(tsim_out[qb * P : (qb + 1) * P, :], vals[:])

                    posf = xp.tile([P, K], f32, tag="posf")
                    nc.vector.tensor_copy(posf[:], pos[:])

                    # softmax weights
                    nmx = xp.tile([P, 1], f32, tag="nmx")
                    nc.vector.tensor_scalar_mul(nmx[:], vals[:, 0:1], -1.0)
                    ex = xp.tile([P, K], f32, tag="ex")
                    nc.scalar.activation(out=ex[:], in_=vals[:], func=mybir.ActivationFunctionType.Exp, bias=nmx[:, 0:1])
                    z = xp.tile([P, 1], f32, tag="z")
                    nc.vector.tensor_reduce(out=z[:], in_=ex[:], axis=mybir.AxisListType.X, op=mybir.AluOpType.add)
                    rz = xp.tile([P, 1], f32, tag="rz")
                    nc.vector.reciprocal(rz[:], z[:])
                    w = sm.tile([P, K], f32, tag=f"w{qb}")
                    nc.scalar.activation(out=w[:], in_=ex[:], func=mybir.ActivationFunctionType.Copy, scale=rz[:, 0:1])

                    # slot = chunk[c]*CH + pos - CH*c where c = pos // CH,
                    # one-hot over the 32 selected chunks (exact f32 integer math)
                    a = xp.tile([P, K, K], f32, tag="cmpa")
                    nc.vector.tensor_tensor(
                        out=a[:],
                        in0=posf[:].rearrange("p (j o) -> p j o", o=1).to_broadcast([P, K, K]),
                        in1=iCH[:].rearrange("p (j c) -> p j c", j=1).to_broadcast([P, K, K]),
                        op=mybir.AluOpType.is_ge,
                    )
                    b = xp.tile([P, K, K], f32, tag="cmpb")
                    nc.vector.tensor_tensor(
                        out=b[:],
                        in0=posf[:].rearrange("p (j o) -> p j o", o=1).to_broadcast([P, K, K]),
                        in1=iCH[:].rearrange("p (j c) -> p j c", j=1).to_broadcast([P, K, K]),
                        op=mybir.AluOpType.is_lt,
                    )
                    chs = xp.tile([P, K, K], f32, tag="chs")
                    nc.vector.tensor_tensor(out=chs[:, :, : K - 1], in0=a[:, :, : K - 1], in1=b[:, :, 1:], op=mybir.AluOpType.mult)
                    nc.vector.tensor_copy(chs[:, :, K - 1 : K], a[:, :, K - 1 : K])
                    t1 = xp.tile([P, K, K], f32, tag="t1")
                    nc.vector.tensor_tensor(
                        out=t1[:], in0=chs[:],
                        in1=iCH[:].rearrange("p (j c) -> p j c", j=1).to_broadcast([P, K, K]),
                        op=mybir.AluOpType.mult,
                    )
                    cCH = xp.tile([P, K], f32, tag="cCH")
                    nc.vector.tensor_reduce(out=cCH[:], in_=t1[:], axis=mybir.AxisListType.X, op=mybir.AluOpType.add)
                    nc.vector.tensor_tensor(
                        out=t1[:], in0=chs[:],
                        in1=cidxf_t[qb][:].rearrange("p (j c) -> p j c", j=1).to_broadcast([P, K, K]),
                        op=mybir.AluOpType.mult,
                    )
                    chunkv = xp.tile([P, K], f32, tag="chunkv")
                    nc.vector.tensor_reduce(out=chunkv[:], in_=t1[:], axis=mybir.AxisListType.X, op=mybir.AluOpType.add)
                    slot = xp.tile([P, K], f32, tag="slot")
                    nc.vector.tensor_scalar_mul(slot[:], chunkv[:], float(CH))
                    nc.vector.tensor_add(slot[:], slot[:], posf[:])
                    nc.vector.tensor_sub(slot[:], slot[:], cCH[:])
                    sloti = sm.tile([P, K], mybir.dt.int32, tag=f"sloti{qb}")
                    nc.vector.tensor_copy(sloti[:], slot[:])

                    # gather memory rows; weighted sum in groups of 8
                    retr = mg.tile([P, D], f32, tag="retr")
                    for jg in range(K // 8):
                        g = mg.tile([P, 8, D], f32, tag="g")
                        for j in range(8):
                            nc.gpsimd.indirect_dma_start(
                                out=g[:, j, :], out_offset=None,
                                in_=mem[:],
                                in_offset=bass.IndirectOffsetOnAxis(ap=sloti[:, jg * 8 + j : jg * 8 + j + 1], axis=0),
                            )
                        for j in range(8):
                            nc.scalar.activation(
                                out=g[:, j, :], in_=g[:, j, :],
                                func=mybir.ActivationFunctionType.Copy,
                                scale=w[:, jg * 8 + j : jg * 8 + j + 1],
                            )
                        part = mg.tile([P, D], f32, tag="part")
                        nc.vector.tensor_reduce(
                            out=part[:], in_=g[:].rearrange("p j d -> p d j"),
                            axis=mybir.AxisListType.X, op=mybir.AluOpType.add,
                        )
                        if jg == 0:
                            nc.vector.tensor_copy(retr[:], part[:])
                        else:
                            nc.vector.tensor_add(retr[:], retr[:], part[:])
                    nc.sync.dma_start(retr_out[qb * P : (qb + 1) * P, :], retr[:])

    nc.finalize()
    _split_waits(nc, limit=1)
    return nc


def kernel(query, memory, W_key, top_k):
    assert int(top_k) == K
    query = np.ascontiguousarray(np.asarray(query, dtype=np.float32))
    memory = np.ascontiguousarray(np.asarray(memory, dtype=np.float32))
    W_key = np.asarray(W_key, dtype=np.float32)
    assert query.shape == (NCORES * BC, D) and memory.shape == (N, D)

    if "nc" not in _cache:
        _cache["nc"] = _build()
    nc = _cache["nc"]

    WT = np.ascontiguousarray(W_key.T)
    memT = np.ascontiguousarray(memory.T)

    def round12(x):
        # the PE's fp32r input format: round-to-nearest, 12-bit mantissa
        mant, exp = np.frexp(x)
        return np.ldexp(np.round(mant * 4096.0) / np.float32(4096.0), exp).astype(np.float32)

    memTr = round12(memT)
    memTres = round12(memT - memTr)
    in_maps = []
    for c in range(NCORES):
        qT = np.ascontiguousarray(query[c * BC : (c + 1) * BC].T)
        in_maps.append({"qT": qT, "WT": WT, "memTr": memTr, "memTres": memTres, "mem": memory})

    import os

    trace = os.environ.get("BASS_TRACE_KERNEL") == "1"
    res = bass_utils.run_bass_kernel_spmd(
        nc, in_maps, core_ids=list(range(NCORES)), trace=trace
    )
    _cache["last_results"] = res
    retrieved = np.concatenate([res.results[c]["retrieved"] for c in range(NCORES)], axis=0)
    top_sim = np.concatenate([res.results[c]["top_sim"] for c in range(NCORES)], axis=0)
    return retrieved, top_sim


# revision 20
# speedup vs baseline: 1.4287x; 1.0874x over previous
"""BTSP memory-bank retrieval kernel for 8 Trainium2 NeuronCores.

Problem: query (4096,1024) f32, memory (65536,1024) f32 unit-norm rows,
W_key (1024,1024) f32, top_k=32.
  query_proj = query @ W_key.T ; qn = l2norm(query_proj)
  sim = qn @ memory.T ; top_sim, top_idx = top_k(sim, 32)
  weights = softmax(top_sim) ; retrieved = weights . memory[top_idx]
Returns (retrieved (4096,1024) f32, top_sim (4096,32) f32).

Sharding: pure data-parallel over queries (B) — each of the 8 cores handles
512 queries against the full memory bank; no collectives. Inside one core:
  - fp32 matmul (PE) computes sim in 512-column tiles, streamed over N
  - per-row chunk maxima (chunks of 128) feed a top-32-chunks tournament:
    the 32nd-largest chunk-max is a valid threshold (32 distinct elements
    are >= it), so the true top-32 elements all live in the 32 selected
    chunks; gather those 4096 candidates and take an exact top-32 with the
    DVE max8/max_index/match_replace instructions
  - softmax over the 32 values, indirect-DMA gather of the 32 memory rows,
    weighted sum -> retrieved.
"""
import numpy as np
import concourse.bass as bass
import concourse.mybir as mybir
from concourse.tile import TileContext
from concourse import bass_utils
from concourse.masks import make_identity

P = 128          # partitions
D = 1024         # feature dim
N = 65536        # memory slots
K = 32           # top-k
BC = 512         # queries per core
NQB = BC // P    # query blocks per core (4)
NT = 512         # sim tile width (N columns per PSUM tile)
NTILES = N // NT # 128
CH = 64          # chunk width for chunk-max tournament
CPT = NT // CH   # chunks per sim tile (8)
NCHUNK = N // CH # chunks per row (1024)
NCORES = 8
NEG = -1e30

_cache = {}


def _split_waits(nc, limit=1):
    """Walrus in this container accepts at most `limit` semaphore waits per
    instruction; move excess waits onto preceding same-engine NOPs."""
    n_split = 0
    for f in nc.m.functions:
        for bb in f.blocks:
            new_list = []
            changed = False
            for ins in bb.instructions:
                si = getattr(ins, "sync_info", None)
                waits = list(si.on_wait) if (si is not None and si.on_wait) else []
                if len(waits) > limit:
                    changed = True
                    n_split += 1
                    extra, keep = waits[:-limit], waits[-limit:]
                    for j in range(0, len(extra), limit):
                        nop = mybir.InstNoOp(
                            name=nc.get_next_instruction_name(), ins=[], outs=[]
                        )
                        nop.engine = ins.engine
                        nop.sync_info = mybir.SyncInfo(
                            on_wait=extra[j : j + limit], on_update=[]
                        )
                        new_list.append(nop)
                    si.on_wait = keep
                new_list.append(ins)
            if changed:
                bb.instructions[:] = new_list
    return n_split


def _build():
    f32 = mybir.dt.float32
    nc = bass.Bass(trn_type="TRN2")

    f32r = mybir.dt.float32r
    qT = nc.dram_tensor("qT", [D, BC], f32, kind="ExternalInput")        # query shard, transposed
    WT = nc.dram_tensor("WT", [D, D], f32, kind="ExternalInput")         # W_key.T ([din, dout])
    # memory transposed, split on host into 12-bit-mantissa value + residual
    # (the PE's fp32r format): memT ~= memTr + memTres exactly to ~2^-24.
    memTr = nc.dram_tensor("memTr", [D, N], f32r, kind="ExternalInput")
    memTres = nc.dram_tensor("memTres", [D, N], f32r, kind="ExternalInput")
    mem = nc.dram_tensor("mem", [N, D], f32, kind="ExternalInput")       # memory (row gather)

    retr_out = nc.dram_tensor("retrieved", [BC, D], f32, kind="ExternalOutput")
    tsim_out = nc.dram_tensor("top_sim", [BC, K], f32, kind="ExternalOutput")

    with TileContext(nc) as tc:
        with (
            tc.tile_pool(name="persist", bufs=1) as pp,
            tc.tile_pool(name="dram", bufs=1, space="DRAM") as dp,
        ):
            # ---------------- phase 0: query proj + l2norm + transpose -----
            qnT_r = pp.tile([P, NQB, 8, P], f32r)    # R12(qnT)
            qnT_rr = pp.tile([P, NQB, 8, P], f32r)   # R12(qnT - R12(qnT))
            ident = pp.tile([P, P], f32)
            make_identity(nc, ident[:])
            M = pp.tile([P, NQB, NCHUNK], f32)     # chunk maxima per query block

            with (
                tc.tile_pool(name="proj_sb", bufs=2) as sb0,
                tc.tile_pool(name="proj_ps", bufs=2, space="PSUM") as ps0,
                tc.tile_pool(name="tr_ps", bufs=2, space="PSUM") as ps0t,
            ):
                qt_all = sb0.tile([P, 8, NQB, P], f32, tag="qt")   # [din128, k, qb, q]
                nc.sync.dma_start(
                    qt_all[:], qT[:].rearrange("(k p) (qb q) -> p k qb q", p=P, q=P)
                )
                wt_all = sb0.tile([P, 8, D], f32, tag="wt")        # [din128, k, dout]
                nc.sync.dma_start(wt_all[:], WT[:].rearrange("(k p) d -> p k d", p=P))

                for qb in range(NQB):
                    qp = sb0.tile([P, D], f32, tag="qp")           # [q, dout]
                    for half in range(2):
                        psq = ps0.tile([P, NT], f32, space="PSUM", tag="psq")
                        for k in range(8):
                            nc.tensor.matmul(
                                psq[:],
                                qt_all[:, k, qb, :],
                                wt_all[:, k, half * NT : (half + 1) * NT],
                                start=(k == 0),
                                stop=(k == 7),
                            )
                        nc.vector.tensor_copy(qp[:, half * NT : (half + 1) * NT], psq[:])
                    sq = sb0.tile([P, D], f32, tag="sq")
                    nc.vector.tensor_tensor(out=sq[:], in0=qp[:], in1=qp[:], op=mybir.AluOpType.mult)
                    nrm2 = sb0.tile([P, 1], f32, tag="n2")
                    nc.vector.tensor_reduce(out=nrm2[:], in_=sq[:], axis=mybir.AxisListType.X, op=mybir.AluOpType.add)
                    nrm = sb0.tile([P, 1], f32, tag="nr")
                    nc.scalar.activation(out=nrm[:], in_=nrm2[:], func=mybir.ActivationFunctionType.Sqrt)
                    nc.vector.tensor_scalar_max(nrm[:], nrm[:], 1e-12)
                    rn = sb0.tile([P, 1], f32, tag="rn")
                    nc.vector.reciprocal(rn[:], nrm[:])
                    qn = sb0.tile([P, D], f32, tag="qn")
                    nc.scalar.activation(out=qn[:], in_=qp[:], func=mybir.ActivationFunctionType.Copy, scale=rn[:, 0:1])
                    # transpose 128x128 blocks, then split into fp32r value+residual
                    for k in range(8):
                        pst = ps0t.tile([P, P], f32, space="PSUM", tag="pst")
                        nc.tensor.transpose(pst[:], qn[:, k * P : (k + 1) * P], ident[:])
                        qnt_f = sb0.tile([P, P], f32, tag="qntf")
                        nc.vector.tensor_copy(qnt_f[:], pst[:])
                        nc.vector.tensor_copy(qnT_r[:, qb, k, :], qnt_f[:])
                        resid = sb0.tile([P, P], f32, tag="resid")
                        nc.vector.tensor_sub(
                            resid[:], qnt_f[:], qnT_r[:, qb, k, :].bitcast(f32)
                        )
                        nc.vector.tensor_copy(qnT_rr[:, qb, k, :], resid[:])

            # ---------------- phase 1: sim matmul + chunk maxima -----------
            # one DRAM scratch per query block: indirect-DMA sources need offset 0
            sim_dram = [
                dp.tile([P * NCHUNK, CH], f32, name=f"sim_scratch{i}", tag=f"sim_scratch{i}")
                for i in range(NQB)
            ]
            with (
                tc.tile_pool(name="mem_sb", bufs=3) as mp,
                tc.tile_pool(name="sim_sb", bufs=4) as sp,
                tc.tile_pool(name="sim_ps", bufs=2, space="PSUM") as psp,
            ):
                for nt in range(NTILES):
                    mtr = mp.tile([P, 8, NT], f32r, tag="mtr")
                    nc.sync.dma_start(
                        mtr[:], memTr[:, nt * NT : (nt + 1) * NT].rearrange("(k p) n -> p k n", p=P)
                    )
                    mtres = mp.tile([P, 8, NT], f32r, tag="mtres")
                    nc.sync.dma_start(
                        mtres[:], memTres[:, nt * NT : (nt + 1) * NT].rearrange("(k p) n -> p k n", p=P)
                    )
                    for qb in range(NQB):
                        ps = psp.tile([P, NT], f32, space="PSUM", tag=f"ps{qb}")
                        # sim = R(q)R(m) + R(q)R(m_res) + R(q_res)R(m)
                        # (same-weight matmuls paired for weight-load reuse)
                        for k in range(8):
                            nc.tensor.matmul(
                                ps[:], qnT_r[:, qb, k, :], mtr[:, k, :],
                                start=(k == 0), stop=False,
                            )
                            nc.tensor.matmul(
                                ps[:], qnT_r[:, qb, k, :], mtres[:, k, :],
                                start=False, stop=False,
                            )
                        for k in range(8):
                            nc.tensor.matmul(
                                ps[:], qnT_rr[:, qb, k, :], mtr[:, k, :],
                                start=False, stop=(k == 7),
                            )
                        sim_sb = sp.tile([P, NT], f32, tag="sim")
                        nc.vector.tensor_copy(sim_sb[:], ps[:])
                        # chunk maxima -> M
                        nc.vector.tensor_reduce(
                            out=M[:, qb, nt * CPT : (nt + 1) * CPT],
                            in_=sim_sb[:].rearrange("p (c e) -> p c e", e=CH),
                            axis=mybir.AxisListType.X,
                            op=mybir.AluOpType.max,
                        )
                        # spill sim tile: rows q*512+chunk of width 128
                        nc.sync.dma_start(
                            sim_dram[qb][:]
                            .rearrange("(q c) e -> q c e", q=P)[:, nt * CPT : (nt + 1) * CPT, :],
                            sim_sb[:].rearrange("p (c e) -> p c e", e=CH),
                        )

            # ---------------- phases 2-5: chunk select, gather, exact top-k ------
            with (
                tc.tile_pool(name="sel_sb", bufs=2) as xp,
                tc.tile_pool(name="sml_sb", bufs=1) as sm,
                tc.tile_pool(name="gat_sb", bufs=2) as gp,
                tc.tile_pool(name="mg_sb", bufs=2) as mg,
            ):
                iota_pN = pp.tile([P, K], mybir.dt.int32)     # p*NCHUNK
                nc.gpsimd.iota(iota_pN[:], pattern=[[0, K]], base=0, channel_multiplier=NCHUNK)
                iota_k32 = pp.tile([P, K], mybir.dt.int32)    # 0..31
                nc.gpsimd.iota(iota_k32[:], pattern=[[1, K]], base=0, channel_multiplier=0)
                iota_k32f = pp.tile([P, K], f32)
                nc.vector.tensor_copy(iota_k32f[:], iota_k32[:])
                iCH = pp.tile([P, K], f32)                    # CH * (0..31)
                nc.vector.tensor_scalar_mul(iCH[:], iota_k32f[:], float(CH))

                cidxf_t, goff_t = {}, {}
                # stage A: top-32 chunks per block (DVE) + gather offsets
                for qb in range(NQB):
                    Mw = xp.tile([P, NCHUNK], f32, tag="Mw")
                    nc.vector.tensor_copy(Mw[:], M[:, qb, :])
                    cvals = xp.tile([P, K], f32, tag="cvals")
                    cidx = xp.tile([P, K], mybir.dt.uint32, tag="cidx")
                    for r in range(4):
                        nc.vector.max(out=cvals[:, r * 8 : (r + 1) * 8], in_=Mw[:])
                        nc.vector.max_index(
                            out=cidx[:, r * 8 : (r + 1) * 8],
                            in_max=cvals[:, r * 8 : (r + 1) * 8], in_values=Mw[:],
                        )
                        if r < 3:
                            nc.vector.match_replace(
                                out=Mw[:], in_to_replace=cvals[:, r * 8 : (r + 1) * 8],
                                in_values=Mw[:], imm_value=NEG,
                            )
                    cidxf = sm.tile([P, K], f32, tag=f"cidxf{qb}")
                    nc.vector.tensor_copy(cidxf[:], cidx[:])
                    cidxf_t[qb] = cidxf
                    goff = sm.tile([P, K], mybir.dt.int32, tag=f"goff{qb}")
                    nc.vector.tensor_tensor(
                        out=goff[:], in0=iota_pN[:],
                        in1=cidx[:].bitcast(mybir.dt.int32), op=mybir.AluOpType.add,
                    )
                    goff_t[qb] = goff

                # stages B-D per block, pipelined via bufs=2 pools
                for qb in range(NQB):
                    gsim = gp.tile([P, K, CH], f32, tag="gsim")
                    for j in range(K):
                        nc.gpsimd.indirect_dma_start(
                            out=gsim[:, j, :], out_offset=None,
                            in_=sim_dram[qb][:],
                            in_offset=bass.IndirectOffsetOnAxis(ap=goff_t[qb][:, j : j + 1], axis=0),
                        )

                    gflat = gsim[:].rearrange("p k e -> p (k e)")
                    vals = sm.tile([P, K], f32, tag=f"vals{qb}")
                    pos = xp.tile([P, K], mybir.dt.uint32, tag="pos")
                    for r in range(4):
                        nc.vector.max(out=vals[:, r * 8 : (r + 1) * 8], in_=gflat)
                        nc.vector.max_index(
                            out=pos[:, r * 8 : (r + 1) * 8],
                            in_max=vals[:, r * 8 : (r + 1) * 8], in_values=gflat,
                        )
                        if r < 3:
                            nc.vector.match_replace(
                                out=gflat, in_to_replace=vals[:, r * 8 : (r + 1) * 8],
                                in_values=gflat, imm_value=NEG,
                            )
                    nc.sync.dma_start(tsim_out[qb * P : (qb + 1) * P, :], vals[:])

                    posf = xp.tile([P, K], f32, tag="posf")
                    nc.vector.tensor_copy(posf[:], pos[:])

                    # softmax weights
                    nmx = xp.tile([P, 1], f32, tag="nmx")
                    nc.vector.tensor_scalar_mul(nmx[:], vals[:, 0:1], -1.0)
                    ex = xp.tile([P, K], f32, tag="ex")
                    nc.scalar.activation(out=ex[:], in_=vals[:], func=mybir.ActivationFunctionType.Exp, bias=nmx[:, 0:1])
                    z = xp.tile([P, 1], f32, tag="z")
                    nc.vector.tensor_reduce(out=z[:], in_=ex[:], axis=mybir.AxisListType.X, op=mybir.AluOpType.add)
                    rz = xp.tile([P, 1], f32, tag="rz")
                    nc.vector.reciprocal(rz[:], z[:])
                    w = sm.tile([P, K], f32, tag=f"w{qb}")
                    nc.scalar.activation(out=w[:], in_=ex[:], func=mybir.ActivationFunctionType.Copy, scale=rz[:, 0:1])

                    # slot = chunk[c]*CH + pos - CH*c where c = pos // CH,
                    # one-hot over the 32 selected chunks (exact f32 integer math)
                    a = xp.tile([P, K, K], f32, tag="cmpa")
                    nc.vector.tensor_tensor(
                        out=a[:],
                        in0=posf[:].rearrange("p (j o) -> p j o", o=1).to_broadcast([P, K, K]),
                        in1=iCH[:].rearrange("p (j c) -> p j c", j=1).to_broadcast([P, K, K]),
                        op=mybir.AluOpType.is_ge,
                    )
                    b = xp.tile([P, K, K], f32, tag="cmpb")
                    nc.vector.tensor_tensor(
                        out=b[:],
                        in0=posf[:].rearrange("p (j o) -> p j o", o=1).to_broadcast([P, K, K]),
                        in1=iCH[:].rearrange("p (j c) -> p j c", j=1).to_broadcast([P, K, K]),
                        op=mybir.AluOpType.is_lt,
                    )
                    chs = xp.tile([P, K, K], f32, tag="chs")
                    nc.vector.tensor_tensor(out=chs[:, :, : K - 1], in0=a[:, :, : K - 1], in1=b[:, :, 1:], op=mybir.AluOpType.mult)
                    nc.vector.tensor_copy(chs[:, :, K - 1 : K], a[:, :, K - 1 : K])
                    t1 = xp.tile([P, K, K], f32, tag="t1")
                    nc.vector.tensor_tensor(
                        out=t1[:], in0=chs[:],
                        in1=iCH[:].rearrange("p (j c) -> p j c", j=1).to_broadcast([P, K, K]),
                        op=mybir.AluOpType.mult,
                    )
                    cCH = xp.tile([P, K], f32, tag="cCH")
                    nc.vector.tensor_reduce(out=cCH[:], in_=t1[:], axis=mybir.AxisListType.X, op=mybir.AluOpType.add)
                    nc.vector.tensor_tensor(
                        out=t1[:], in0=chs[:],
                        in1=cidxf_t[qb][:].rearrange("p (j c) -> p j c", j=1).to_broadcast([P, K, K]),
                        op=mybir.AluOpType.mult,
                    )
                    chunkv = xp.tile([P, K], f32, tag="chunkv")
                    nc.vector.tensor_reduce(out=chunkv[:], in_=t1[:], axis=mybir.AxisListType.X, op=mybir.AluOpType.add)
                    slot = xp.tile([P, K], f32, tag="slot")
                    nc.vector.tensor_scalar_mul(slot[:], chunkv[:], float(CH))
                    nc.vector.tensor_add(slot[:], slot[:], posf[:])
                    nc.vector.tensor_sub(slot[:], slot[:], cCH[:])
                    sloti = sm.tile([P, K], mybir.dt.int32, tag=f"sloti{qb}")
                    nc.vector.tensor_copy(sloti[:], slot[:])

                    # gather memory rows; weighted sum in groups of 8
                    retr = mg.tile([P, D], f32, tag="retr")
                    for jg in range(K // 8):
                        g = mg.tile([P, 8, D], f32, tag="g")
                        for j in range(8):
                            nc.gpsimd.indirect_dma_start(
                                out=g[:, j, :], out_offset=None,
                                in_=mem[:],
                                in_offset=bass.IndirectOffsetOnAxis(ap=sloti[:, jg * 8 + j : jg * 8 + j + 1], axis=0),
                            )
                        for j in range(8):
                            nc.scalar.activation(
                                out=g[:, j, :], in_=g[:, j, :],
                                func=mybir.ActivationFunctionType.Copy,
                                scale=w[:, jg * 8 + j : jg * 8 + j + 1],
                            )
                        # pairwise tree-sum over the 8 scaled rows (unit-stride adds)
                        nc.vector.tensor_add(g[:, 0:4, :], g[:, 0:4, :], g[:, 4:8, :])
                        nc.vector.tensor_add(g[:, 0:2, :], g[:, 0:2, :], g[:, 2:4, :])
                        if jg == 0:
                            nc.vector.tensor_add(retr[:], g[:, 0, :], g[:, 1, :])
                        else:
                            nc.vector.tensor_add(g[:, 0, :], g[:, 0, :], g[:, 1, :])
                            nc.vector.tensor_add(retr[:], retr[:], g[:, 0, :])
                    nc.sync.dma_start(retr_out[qb * P : (qb + 1) * P, :], retr[:])

    nc.finalize()
    _split_waits(nc, limit=1)
    return nc


def kernel(query, memory, W_key, top_k):
    assert int(top_k) == K
    query = np.ascontiguousarray(np.asarray(query, dtype=np.float32))
    memory = np.ascontiguousarray(np.asarray(memory, dtype=np.float32))
    W_key = np.asarray(W_key, dtype=np.float32)
    assert query.shape == (NCORES * BC, D) and memory.shape == (N, D)

    if "nc" not in _cache:
        _cache["nc"] = _build()
    nc = _cache["nc"]

    WT = np.ascontiguousarray(W_key.T)
    memT = np.ascontiguousarray(memory.T)

    def round12(x):
        # the PE's fp32r input format: round-to-nearest, 12-bit mantissa
        mant, exp = np.frexp(x)
        return np.ldexp(np.round(mant * 4096.0) / np.float32(4096.0), exp).astype(np.float32)

    memTr = round12(memT)
    memTres = round12(memT - memTr)
    in_maps = []
    for c in range(NCORES):
        qT = np.ascontiguousarray(query[c * BC : (c + 1) * BC].T)
        in_maps.append({"qT": qT, "WT": WT, "memTr": memTr, "memTres": memTres, "mem": memory})

    import os

    trace = os.environ.get("BASS_TRACE_KERNEL") == "1"
    res = bass_utils.run_bass_kernel_spmd(
        nc, in_maps, core_ids=list(range(NCORES)), trace=trace
    )
    _cache["last_results"] = res
    retrieved = np.concatenate([res.results[c]["retrieved"] for c in range(NCORES)], axis=0)
    top_sim = np.concatenate([res.results[c]["top_sim"] for c in range(NCORES)], axis=0)
    return retrieved, top_sim
